# revision 24
# baseline (speedup 1.0000x reference)
"""Causal self-attention Trainium2 Bass kernel (v4).

Problem (hardcoded): B=4, S=2048, D=1024, H=16 heads, head_dim=64, fp32.
    qkv = x @ W_attn + b_attn; causal softmax attention; y @ W_proj + b_proj.

Sharding over 8 NeuronCores: core c -> (batch b = c//2, head-group g = c%2).
All matmul operands are bf16 (PSUM accumulation stays fp32); x^T lives
resident in SBUF (32KB/partition) so V s-tiles are computed on demand inside
the attention loop. Per core, for its batch and its 8 heads (512 features):
    Q^T, K^T [512f, 2048s] bf16 via N=1024 PSUM chains + ACT bias-drain;
    the second half of the QK chains is interleaved into attention chunks
    0-1 so the PE never idles between "phases".
    flash-style causal attention in transposed layout:
        scores^T [128k, 512q] tiles = K^T.T @ Q^T (per head, diag-trimmed)
        expS = exp(scores/8) via ACT -> bf16
        out^T [128, 512q] += [V | ones64].T @ expS  (rows 64:128 of the
            accumulator hold the softmax denominator replicated 64x, so
            normalization is DVE-only: copy + reciprocal + multiply)
    partial = y^T.T @ W_proj[group rows] -> [2048, 1024]; projection for
    chunk j is emitted one head-pair late so it never waits on the last
    pair's normalization.
Host: out[b] = partial(core 2b) + partial(core 2b+1) + b_proj + b_attn_v @ W_proj.
q/k biases are applied on-device (per-partition ACT bias); the v bias commutes
through softmax (rows sum to 1) so its projection is added on the host.
"""
import sys
if '/opt/trn_rl_repo' not in sys.path:
    sys.path.insert(0, '/opt/trn_rl_repo')

import numpy as np
import ml_dtypes
import concourse.bass as bass
import concourse.mybir as mybir
import concourse.tile as tile
from concourse import bacc
from concourse import bass_utils

F32 = mybir.dt.float32
BF16 = mybir.dt.bfloat16
AF = mybir.ActivationFunctionType
ALU = mybir.AluOpType

B, S, D, H, HD = 4, 2048, 1024, 16, 64
NCORES = 8
FPC = 512            # feature dims per core (8 heads * 64)
NPAIR = 4            # head pairs per core
DC = D // 128        # 8 contraction chunks for QKV/proj of x
NST = S // 128       # 16 s-tiles

_CACHE = {}


def _build_program():
    nc = bacc.Bacc("TRN2", target_bir_lowering=False, debug=False,
                   enable_asserts=False, num_devices=NCORES)

    xT_d = nc.dram_tensor("xT", [D, S], BF16, kind="ExternalInput").ap()
    wq_d = nc.dram_tensor("wq", [D, FPC], BF16, kind="ExternalInput").ap()
    wk_d = nc.dram_tensor("wk", [D, FPC], BF16, kind="ExternalInput").ap()
    wv_d = nc.dram_tensor("wv", [D, FPC], BF16, kind="ExternalInput").ap()
    wp_d = nc.dram_tensor("wp", [FPC, D], BF16, kind="ExternalInput").ap()
    bq_d = nc.dram_tensor("bq", [FPC], F32, kind="ExternalInput").ap()
    bk_d = nc.dram_tensor("bk", [FPC], F32, kind="ExternalInput").ap()
    out_d = nc.dram_tensor("out", [S, D], F32, kind="ExternalOutput").ap()

    from contextlib import ExitStack
    with tile.TileContext(nc) as tc, ExitStack() as ctx:
        persist = ctx.enter_context(tc.tile_pool(name="persist", bufs=1))
        QT = [persist.tile([128, S], BF16, name=f"qt{p}") for p in range(NPAIR)]
        KT = [persist.tile([128, S], BF16, name=f"kt{p}") for p in range(NPAIR)]
        yT = [persist.tile([128, S], BF16, name=f"yt{p}") for p in range(NPAIR)]
        # V tiles: [128 s, 8 heads, 128] bf16 -- cols 64:128 are ones so the
        # PV matmul replicates the denominator across PSUM rows 64:128
        Vt = [persist.tile([128, 8, 128], BF16, name=f"v{i}") for i in range(NST)]
        x_sb = persist.tile([128, DC, S], BF16, name="x_sb")
        wq_sb = persist.tile([128, DC, FPC], BF16, name="wq_sb")
        wk_sb = persist.tile([128, DC, FPC], BF16, name="wk_sb")
        wv_sb = persist.tile([128, DC, FPC], BF16, name="wv_sb")
        wp_sb = persist.tile([128, 4, D], BF16, name="wp_sb")
        bq_sb = persist.tile([128, 4], F32, name="bq_sb")
        bk_sb = persist.tile([128, 4], F32, name="bk_sb")

        expool = ctx.enter_context(tc.tile_pool(name="expool", bufs=4))
        smpool = ctx.enter_context(tc.tile_pool(name="smpool", bufs=2))
        outsb = ctx.enter_context(tc.tile_pool(name="outsb", bufs=3))
        ring = ctx.enter_context(tc.tile_pool(name="ring", bufs=2, space="PSUM"))
        pvps = ctx.enter_context(tc.tile_pool(name="pvps", bufs=2, space="PSUM"))

        # Input DMAs are issue-rate-bound (~650ns each), so split them over
        # BOTH hardware DMA-issue engines: Sync carries biases + x (the
        # first-chain gate), the idle Activation engine carries all weights
        # in parallel. Biases go first -- they gate every QK ACT-drain.
        nc.sync.dma_start(bq_sb[:], bq_d.rearrange("(c p) -> p c", p=128))
        nc.sync.dma_start(bk_sb[:], bk_d.rearrange("(c p) -> p c", p=128))
        for c in range(DC):
            nc.scalar.dma_start(wq_sb[:, c, :], wq_d[128 * c:128 * c + 128, :])
        for c in range(DC):
            nc.sync.dma_start(x_sb[:, c, 0:512],
                              xT_d[128 * c:128 * c + 128, 0:512])
        for c in range(DC):
            nc.sync.dma_start(x_sb[:, c, 512:1024],
                              xT_d[128 * c:128 * c + 128, 512:1024])
        for c in range(DC):
            nc.sync.dma_start(wk_sb[:, c, :], wk_d[128 * c:128 * c + 128, :])
        for c in range(DC):
            nc.sync.dma_start(x_sb[:, c, 1024:2048],
                              xT_d[128 * c:128 * c + 128, 1024:2048])
        for c in range(DC):
            nc.sync.dma_start(wv_sb[:, c, :], wv_d[128 * c:128 * c + 128, :])
        for c in range(4):
            nc.sync.dma_start(wp_sb[:, c, :], wp_d[128 * c:128 * c + 128, :])

        # ones columns of Vt (denominator replication)
        onesv_f = persist.tile([128, 8, 64], F32, name="onesv_f")
        nc.gpsimd.memset(onesv_f[:], 1.0)
        for i in range(NST):
            nc.vector.tensor_copy(Vt[i][:, :, 64:128], onesv_f[:])

        # universal [128,128] strict-upper-triangle mask: 1.0 iff kp <= qf
        tri_f = persist.tile([128, 128], F32, name="tri_f")
        nc.gpsimd.memset(tri_f[:], 1.0)
        nc.gpsimd.affine_select(
            out=tri_f[:], in_=tri_f[:],
            compare_op=ALU.is_ge, fill=0.0,
            base=0, pattern=[[1, 128]], channel_multiplier=-1)
        tri = persist.tile([128, 128], BF16, name="tri")
        nc.vector.tensor_copy(tri[:], tri_f[:])

        def qk_unit(half, f, w_sb, b_sb, dstT):
            # matmul N is capped at 512 fp32 (one PSUM bank): two chains
            # fill the [128,1024] ring tile, drained by a single ACT op
            h0 = 1024 * half
            ps = ring.tile([128, 1024], F32, name=f"qk{half}_{f}_{id(w_sb)}",
                           tag="ring")
            for s in range(2):
                for c in range(DC):
                    nc.tensor.matmul(ps[:, 512 * s:512 * s + 512],
                                     w_sb[:, c, 128 * f:128 * f + 128],
                                     x_sb[:, c, h0 + 512 * s:h0 + 512 * s + 512],
                                     start=(c == 0), stop=(c == DC - 1))
            nc.scalar.activation(dstT[f][:, h0:h0 + 1024], ps[:],
                                 AF.Identity, bias=b_sb[:, f:f + 1])

        def v_unit(i):
            psv = ring.tile([128, 1024], F32, name=f"psv{i}", tag="ring")
            for c in range(DC):
                nc.tensor.matmul(psv[:, 0:512],
                                 x_sb[:, c, 128 * i:128 * i + 128],
                                 wv_sb[:, c, :],
                                 start=(c == 0), stop=(c == DC - 1))
            nc.vector.tensor_copy(
                Vt[i][:, :, 0:64],
                psv[:, 0:512].rearrange("p (h u) -> p h u", h=8))

        def emit_proj(j, lo=0, hi=4, copy_act=False):
            for ii in range(lo, hi):
                i = 4 * j + ii
                po = ring.tile([128, 1024], F32, name=f"po{i}", tag="ring")
                for o in range(2):
                    for p2 in range(NPAIR):
                        nc.tensor.matmul(po[:, 512 * o:512 * o + 512],
                                         yT[p2][:, 128 * i:128 * i + 128],
                                         wp_sb[:, p2, 512 * o:512 * o + 512],
                                         start=(p2 == 0), stop=(p2 == 3))
                ot = outsb.tile([128, 1024], F32, name=f"ot{i}", tag="ot")
                if copy_act:
                    nc.scalar.copy(ot[:], po[:])
                else:
                    nc.vector.tensor_copy(ot[:], po[:])
                nc.sync.dma_start(out_d[128 * i:128 * i + 128, :], ot[:])

        # first QK half upfront; second half interleaved into chunks 0-1
        for f in range(4):
            qk_unit(0, f, wq_sb, bq_sb, QT)
            qk_unit(0, f, wk_sb, bk_sb, KT)
        qk_pending = [(1, f, w, b, d) for f in range(4)
                      for (w, b, d) in ((wq_sb, bq_sb, QT), (wk_sb, bk_sb, KT))]

        # Attention as one software-pipelined stream: the PV of tile T is
        # emitted only after tile T+1's scores+exp, ACROSS pair and chunk
        # boundaries, so the PE always has score matmuls to run while ACT
        # computes the exp the PV is waiting on. Each pair's normalization
        # (and the interleave hooks: second QK half, deferred projection)
        # is emitted right after its last PV flushes.
        st_pend = [None]
        pending_proj = [None]

        def flush_pend():
            if st_pend[0] is not None:
                pv_fn, norm_fn = st_pend[0]
                st_pend[0] = None
                pv_fn()
                if norm_fn is not None:
                    norm_fn()

        def make_norm(j, p, acc2):
            q0 = 512 * j

            def norm_fn():
                # rows 64:128 of acc2 hold both heads' denominators
                # replicated; one DVE copy + reciprocal covers both, then a
                # multiply per head
                den = smpool.tile([64, 1024], F32, name=f"den{j}_{p}",
                                  tag="den")
                nc.vector.tensor_copy(den[:], acc2[64:128, :])
                scr = smpool.tile([64, 1024], F32, name=f"scr{j}_{p}",
                                  tag="scr")
                rec = smpool.tile([64, 1024], F32, name=f"rec{j}_{p}",
                                  tag="rec")
                nc.vector.reciprocal_approx_accurate(rec[:], den[:], scr[:])
                for hi in range(2):
                    ys = yT[p][64 * hi:64 * hi + 64, q0:q0 + 512]
                    nc.vector.tensor_tensor(
                        ys, acc2[0:64, 512 * hi:512 * hi + 512],
                        rec[:, 512 * hi:512 * hi + 512], ALU.mult)
                # second QK half rides inside early attention chunks
                if qk_pending:
                    qk_unit(*qk_pending.pop(0))
                # projection for the previous q-chunk, deferred one pair;
                # two of chunk 2's units are held back to the tail, where
                # they are the only PE work independent of the last pair
                if p == 0 and pending_proj[0] is not None:
                    emit_proj(pending_proj[0],
                              hi=2 if pending_proj[0] == 2 else 4)
                    pending_proj[0] = None
            return norm_fn

        for j in range(4):          # q-chunks of 512
            q0 = 512 * j
            nk = 4 * (j + 1)
            for i in range(4 * j, 4 * j + 4):
                v_unit(i)
            for p in range(NPAIR):
                acc2 = pvps.tile([128, 1024], F32, name=f"acc{j}_{p}",
                                 tag="acc")
                for t in range(nk):
                    k0 = 128 * t
                    oi = t - 4 * j
                    lo = max(0, 128 * oi)
                    sc = ring.tile([128, 1024], F32, name=f"sc{j}_{p}_{t}",
                                   tag="ring")
                    nc.tensor.matmul(sc[:, lo:512],
                                     KT[p][0:64, k0:k0 + 128],
                                     QT[p][0:64, q0 + lo:q0 + 512],
                                     start=True, stop=True)
                    nc.tensor.matmul(sc[:, 512 + lo:1024],
                                     KT[p][64:128, k0:k0 + 128],
                                     QT[p][64:128, q0 + lo:q0 + 512],
                                     start=True, stop=True)
                    ex = expool.tile([128, 1024], BF16, name=f"ex{j}_{p}_{t}",
                                     tag="ex")
                    nc.scalar.activation(ex[:, lo:1024], sc[:, lo:1024],
                                         AF.Exp, scale=0.125)
                    if oi >= 0:   # strict upper triangle of the diag block
                        nc.vector.tensor_tensor(
                            ex[:, lo:lo + 128], ex[:, lo:lo + 128],
                            tri[:], ALU.mult)
                        nc.vector.tensor_tensor(
                            ex[:, 512 + lo:512 + lo + 128],
                            ex[:, 512 + lo:512 + lo + 128],
                            tri[:], ALU.mult)
                    flush_pend()

                    def pv_fn(acc2=acc2, t=t, ex=ex, lo=lo, p=p, nk=nk):
                        nc.tensor.matmul(acc2[:, lo:512], Vt[t][:, 2 * p, :],
                                         ex[:, lo:512],
                                         start=(t == 0), stop=(t == nk - 1))
                        nc.tensor.matmul(acc2[:, 512 + lo:1024],
                                         Vt[t][:, 2 * p + 1, :],
                                         ex[:, 512 + lo:1024],
                                         start=(t == 0), stop=(t == nk - 1))

                    st_pend[0] = (pv_fn,
                                  make_norm(j, p, acc2) if t == nk - 1 else None)
            pending_proj[0] = j
        # final projection, pipelined around the last pair's normalization:
        # all four units' partial chains (pairs 0-2) are pure PE work that
        # needs nothing from pair 3 -- three of them are emitted before the
        # flush (18 matmuls covering the norm's DVE latency), borrowing the
        # two freed acc slots so four accumulators can be live at once.
        def proj3_partial(i, pool):
            po = pool.tile([128, 1024], F32, name=f"po{i}",
                           tag="ring" if pool is ring else "acc")
            for o in range(2):
                for p2 in range(3):
                    nc.tensor.matmul(po[:, 512 * o:512 * o + 512],
                                     yT[p2][:, 128 * i:128 * i + 128],
                                     wp_sb[:, p2, 512 * o:512 * o + 512],
                                     start=(p2 == 0), stop=False,
                                     skip_group_check=True)
            return po

        # split flush: chunk 2's held-back units go between the last PV and
        # the last normalization -- PE work emitted after a norm stalls
        # until that norm completes, so this is the only slot where real
        # matmuls can run concurrently with the final norm's DVE chain
        last_pv, last_norm = st_pend[0]
        st_pend[0] = None
        last_pv()
        emit_proj(2, lo=2, hi=4, copy_act=True)
        last_norm()
        po_open = {}
        po_open[12] = proj3_partial(12, ring)
        po_open[13] = proj3_partial(13, ring)
        po_open[14] = proj3_partial(14, pvps)
        po_open[15] = proj3_partial(15, pvps)
        for ii in range(4):
            i = 12 + ii
            po = po_open.pop(i)
            for o in range(2):
                nc.tensor.matmul(po[:, 512 * o:512 * o + 512],
                                 yT[3][:, 128 * i:128 * i + 128],
                                 wp_sb[:, 3, 512 * o:512 * o + 512],
                                 start=False, stop=True,
                                 skip_group_check=True)
            ot = outsb.tile([128, 1024], F32, name=f"ot{i}", tag="ot")
            if ii % 2 == 0:
                nc.scalar.copy(ot[:], po[:])
            else:
                nc.vector.tensor_copy(ot[:], po[:])
            nc.sync.dma_start(out_d[128 * i:128 * i + 128, :], ot[:])

    nc.compile()
    return nc


def _get_program():
    if "nc" not in _CACHE:
        _CACHE["nc"] = _build_program()
    return _CACHE["nc"]


def kernel(x, W_attn, b_attn, W_proj, b_proj, _trace=False, _trace_cores=None):
    x = np.asarray(x, np.float32)
    W_attn = np.asarray(W_attn, np.float32)
    b_attn = np.asarray(b_attn, np.float32)
    W_proj = np.asarray(W_proj, np.float32)
    b_proj = np.asarray(b_proj, np.float32)

    nc = _get_program()

    bf = ml_dtypes.bfloat16
    in_maps = []
    for c in range(NCORES):
        b, g = divmod(c, 2)
        gc = slice(FPC * g, FPC * g + FPC)
        in_maps.append({
            "xT": np.ascontiguousarray(x[b].T.astype(bf)),
            "wq": np.ascontiguousarray(W_attn[:, 0 * D:1 * D][:, gc].astype(bf)),
            "wk": np.ascontiguousarray(W_attn[:, 1 * D:2 * D][:, gc].astype(bf)),
            "wv": np.ascontiguousarray(W_attn[:, 2 * D:3 * D][:, gc].astype(bf)),
            "wp": np.ascontiguousarray(W_proj[gc, :].astype(bf)),
            "bq": np.ascontiguousarray(b_attn[0 * D:1 * D][gc]),
            "bk": np.ascontiguousarray(b_attn[1 * D:2 * D][gc]),
        })

    kw = {}
    if _trace:
        kw = dict(trace=True, trace_cores=_trace_cores or [0])
    res = bass_utils.run_bass_kernel_spmd(nc, in_maps, core_ids=list(range(NCORES)),
                                          **kw)

    # host-side reduction: v-bias commutes through softmax -> fold via W_proj
    corr = b_proj + b_attn[2 * D:3 * D] @ W_proj
    out = np.empty((B, S, D), np.float32)
    for b in range(B):
        out[b] = res.results[2 * b]["out"] + res.results[2 * b + 1]["out"] + corr

    if _trace:
        kernel._last_results = res
    return out
